# revision 1
# baseline (speedup 1.0000x reference)
"""Trainium2 Bass kernel for nn_GatedMultiAggHead (segment_reduce).

Strategy (SPMD over 8 NeuronCores):
  - b0/b1/b2 are sorted, so each of the 512 segments is a contiguous row range.
    Core k owns segments [64k, 64k+64): no cross-core reduction needed.
  - Host pads each (core, seg) to a common per-seg length L_j (multiple of 128,
    max over cores) so one program serves all 8 cores, and builds TWO bf16
    layouts per rank:
      hB  [128=H, Npad]            (transposed)   -> PE g-matmuls (stationary
                                                     operand), DVE segment max
      hA  [Npad/256, 128, 256]     (node-major, pair-swizzled so every DMA
                                    partition burst is 512B) -> PE gated/plain
                                    segment sums
  - Per 128-node tile, PE computes:
      g column:  matmul(lhsT=B_slice[128H,128n], rhs=Wg[128,1]) -> PSUM col
      [gsum|sum]: matmul(lhsT=A_tile[128n,128H], rhs=[sigma(g)|1][128n,2])
                  accumulated per segment into PSUM [128H, seg, 2]
  - ACT: batched sigmoid over [128, 512] g columns.  DVE: per-seg reduce_max.
  - Mean pool is folded into the Wp projection: mean@Wp1 = diag(1/cnt)@(sum@Wp1),
    applied as a per-partition row scale after the matmul (cnt from host bincount).
  - Tiny replicated head per core on its own 64 segments: Wp chunks, LayerNorm
    via bn_stats, silu = x*sigmoid(x), W1/W2 with small PE transposes.
  - Output [64,1] f32 per core; host concatenates to [512].
"""

import sys

sys.path.insert(0, "/opt/trn_rl_repo")

import numpy as np
import ml_dtypes

BF16 = ml_dtypes.bfloat16

H = 128
TILE = 128
NCORES = 8
B_SEGS = 512
EPS = 1e-5


# ----------------------------------------------------------------------------
# Host-side planning / packing
# ----------------------------------------------------------------------------

class RankPlan:
    def __init__(self, b, ncores, segs):
        b = np.asarray(b, np.int64)
        nseg_total = ncores * segs
        counts = np.bincount(b, minlength=nseg_total).reshape(ncores, segs)
        L = ((counts.max(axis=0) + TILE - 1) // TILE) * TILE
        L = np.maximum(L, TILE).astype(np.int64)
        starts = np.zeros(segs + 1, np.int64)
        starts[1:] = np.cumsum(L)
        npad_raw = int(starts[-1])
        self.counts = counts                      # [ncores, segs] true counts
        self.L = L                                # [segs] padded len (mult of 128)
        self.starts = starts                      # [segs+1] col offsets
        self.npad = ((npad_raw + 255) // 256) * 256
        self.segs = segs
        self.ncores = ncores
        self.ntiles = npad_raw // TILE            # only real (per-seg) tiles
        # tile -> seg map and first/last flags
        self.tile_seg = np.repeat(np.arange(segs), (L // TILE))
        first = np.zeros(self.ntiles, bool)
        first[(starts[:-1] // TILE)] = True
        last = np.zeros(self.ntiles, bool)
        last[(starts[1:] // TILE) - 1] = True
        self.tile_first = first
        self.tile_last = last
        self.seg_bounds_rows = np.searchsorted(b, np.arange(nseg_total + 1))


def _pack_rank(h, b, plan: RankPlan):
    """Returns hB [ncores,128,npad] bf16, hA [ncores,npad//256,128,256] bf16."""
    ncores, segs, npad = plan.ncores, plan.segs, plan.npad
    h16 = np.asarray(h, np.float32).astype(BF16)
    b = np.asarray(b, np.int64)
    sb = plan.seg_bounds_rows
    hB = np.zeros((ncores, H, npad), BF16)
    hA = np.zeros((ncores, npad // 256, H, 256), BF16)
    for k in range(ncores):
        rs, re = int(sb[k * segs]), int(sb[(k + 1) * segs])
        if re > rs:
            local = b[rs:re] - k * segs                       # local seg id per row
            offs = np.arange(re - rs) + rs - sb[b[rs:re]]     # offset within seg
            dst = plan.starts[local] + offs
            bufA = np.zeros((npad, H), BF16)
            bufA[dst] = h16[rs:re]
        else:
            bufA = np.zeros((npad, H), BF16)
        hB[k] = bufA.T
        # pair swizzle: chunk c holds nodes [256c,256c+256); partition p row is
        # [h[256c+p] | h[256c+128+p]] -> 512B contiguous per partition per chunk
        hA[k] = bufA.reshape(npad // 256, 2, H, H).transpose(0, 2, 1, 3).reshape(
            npad // 256, H, 256)
    return hB, hA


# ----------------------------------------------------------------------------
# Device program
# ----------------------------------------------------------------------------

def build_core_program(plans, consts, segs, gbatch=512, a_chunks=8, bcols_target=2048):
    """Build the single-core (SPMD-replicated) Bass program.

    plans: list of RankPlan (len 3)
    consts: dict with host weight arrays (shared across cores)
    Returns (nc, per_core_input_names, shared_input_names)
    """
    import concourse.bacc as bacc
    import concourse.tile as tile
    from concourse import mybir

    f32 = mybir.dt.float32
    bf16 = mybir.dt.bfloat16
    AX = mybir.AxisListType
    AF = mybir.ActivationFunctionType
    OP = mybir.AluOpType

    nranks = len(plans)
    H3 = H * nranks

    # Bacc (not bare Bass): its compile() runs generate_event_semaphores,
    # which splits multi-wait instructions into standalone EventSemaphore
    # waits — the TRN2 ISA allows at most one inline wait per instruction.
    nc = bacc.Bacc(None, name="gmah")

    # --- dram I/O ---
    per_core = {}
    shared = {}

    hB_d, hA_d, recip_d, wg_d = [], [], [], []
    wp_d = []
    for d, p in enumerate(plans):
        hB_d.append(nc.declare_dram_parameter(f"hB{d}", [H, p.npad], bf16, isOutput=False))
        hA_d.append(nc.declare_dram_parameter(f"hA{d}", [p.npad // 256, H, 256], bf16, isOutput=False))
        recip_d.append(nc.declare_dram_parameter(f"recip{d}", [segs, 1], f32, isOutput=False))
        per_core[f"hB{d}"] = None
        per_core[f"hA{d}"] = None
        per_core[f"recip{d}"] = None
        wg_d.append(nc.declare_dram_parameter(f"wg{d}", [H, 1], bf16, isOutput=False))
        shared[f"wg{d}"] = consts[f"wg{d}"]
        wp_d.append(nc.declare_dram_parameter(f"wp{d}", [4, H, H], f32, isOutput=False))
        shared[f"wp{d}"] = consts[f"wp{d}"]

    bp_t = nc.declare_dram_parameter("bp", [segs, H3], f32, isOutput=False)
    gamma_t = nc.declare_dram_parameter("gamma_b", [segs, H3], f32, isOutput=False)
    beta_t = nc.declare_dram_parameter("beta_b", [segs, H3], f32, isOutput=False)
    w1_t = nc.declare_dram_parameter("w1", [nranks, H, H], f32, isOutput=False)
    b1f_t = nc.declare_dram_parameter("b1f_b", [segs, H], f32, isOutput=False)
    w2_t = nc.declare_dram_parameter("w2", [H, 1], f32, isOutput=False)
    id_t = nc.declare_dram_parameter("id64", [segs, segs], f32, isOutput=False)
    for n in ("bp", "gamma_b", "beta_b", "w1", "b1f_b", "w2", "id64"):
        shared[n] = consts[n]

    out_t = nc.declare_dram_parameter("out", [segs, 1], f32, isOutput=True)

    bg_vals = [float(consts[f"bg{d}"]) for d in range(nranks)]
    b2f_val = float(consts["b2f"])

    with tile.TileContext(nc) as tc:
        with (
            tc.tile_pool(name="singles", bufs=1) as singles,
            tc.tile_pool(name="bpool", bufs=8) as bpool,
            tc.tile_pool(name="apool", bufs=8) as apool,
            tc.tile_pool(name="gsb", bufs=2) as gsbpool,
            tc.tile_pool(name="persist", bufs=1) as persist,
            tc.tile_pool(name="headsb", bufs=1) as headsb,
            tc.tile_pool(name="gpsum", bufs=2, space="PSUM") as gpsum,
            tc.tile_pool(name="gspsum", bufs=2, space="PSUM") as gspsum,
            tc.tile_pool(name="hpsum", bufs=1, space="PSUM") as hpsum,
        ):
            # --- load weights ---
            wg_sb, wp_sb, recip_sb = [], [], []
            for d in range(nranks):
                t = singles.tile([H, 1], bf16, tag=f"wg{d}")
                nc.sync.dma_start(t, wg_d[d][:])
                wg_sb.append(t)
                chunks = []
                for c in range(4):
                    t = singles.tile([H, H], f32, tag=f"wp{d}_{c}")
                    nc.sync.dma_start(t, wp_d[d][c])
                    chunks.append(t)
                wp_sb.append(chunks)
                t = singles.tile([segs, 1], f32, tag=f"recip{d}")
                nc.sync.dma_start(t, recip_d[d][:])
                recip_sb.append(t)
            bp_sb = singles.tile([segs, H3], f32, tag="bp")
            nc.sync.dma_start(bp_sb, bp_t[:])
            gamma_sb = singles.tile([segs, H3], f32, tag="gamma")
            nc.sync.dma_start(gamma_sb, gamma_t[:])
            beta_sb = singles.tile([segs, H3], f32, tag="beta")
            nc.sync.dma_start(beta_sb, beta_t[:])
            w1_sb = []
            for c in range(nranks):
                t = singles.tile([H, H], f32, tag=f"w1_{c}")
                nc.sync.dma_start(t, w1_t[c])
                w1_sb.append(t)
            b1f_sb = singles.tile([segs, H], f32, tag="b1f")
            nc.sync.dma_start(b1f_sb, b1f_t[:])
            w2_sb = singles.tile([H, 1], f32, tag="w2")
            nc.sync.dma_start(w2_sb, w2_t[:])
            id_sb = singles.tile([segs, segs], f32, tag="id64")
            nc.sync.dma_start(id_sb, id_t[:])
            eps_sb = singles.tile([segs, 1], f32, tag="eps")
            nc.vector.memset(eps_sb, EPS)
            bg_sb = []
            for d in range(nranks):
                t = singles.tile([H, 1], f32, tag=f"bg{d}")
                nc.vector.memset(t, bg_vals[d])
                bg_sb.append(t)
            b2f_sb = singles.tile([segs, 1], f32, tag="b2f")
            nc.vector.memset(b2f_sb, b2f_val)

            state = persist.tile([segs, H3], f32, tag="state")

            for d, p in enumerate(plans):
                # --- B groups: consecutive segs until >= bcols_target cols ---
                bgroups = []  # (seg_lo, seg_hi, col_lo, col_hi)
                j = 0
                while j < segs:
                    j0, c0 = j, int(p.starts[j])
                    while j < segs and int(p.starts[j + 1]) - c0 < bcols_target:
                        j += 1
                    j = min(j + 1, segs)
                    bgroups.append((j0, j, c0, int(p.starts[j])))
                tile_group = np.zeros(p.ntiles, np.int64)
                for gi, (j0, j1, c0, c1) in enumerate(bgroups):
                    tile_group[c0 // TILE: c1 // TILE] = gi

                maxp = persist.tile([H, segs], f32, tag=f"maxp{d}")
                psum_gs = gspsum.tile([H, segs, 2], f32, tag="gs")

                bg_tiles = {}

                def touch_bgroup(gi, d=d, p=p, bgroups=bgroups, bg_tiles=bg_tiles,
                                 maxp=maxp):
                    if gi in bg_tiles:
                        return bg_tiles[gi]
                    j0, j1, c0, c1 = bgroups[gi]
                    # DMA target is read by exactly one engine (ACT bounce):
                    # the DMA pseudo-instruction has a single inline wait slot,
                    # so its WAR-on-slot-reuse must not fan in from PE+DVE.
                    traw = bpool.tile([H, c1 - c0], bf16, tag="bgrp_raw")
                    nc.gpsimd.dma_start(traw, hB_d[d][:, c0:c1])
                    t = bpool.tile([H, c1 - c0], bf16, tag="bgrp")
                    nc.scalar.copy(t, traw)
                    # segment maxes (DVE) read from the bounced tile
                    for j in range(j0, j1):
                        s0 = int(p.starts[j]) - c0
                        s1 = int(p.starts[j + 1]) - c0
                        nc.vector.reduce_max(
                            out=maxp[:, j:j + 1], in_=t[:, s0:s1], axis=AX.X)
                    bg_tiles[gi] = t
                    return t

                a_tiles = {}

                def touch_aslab(si, d=d, p=p, a_tiles=a_tiles):
                    # slab si covers chunks [a_chunks*si, ...) i.e. 2*a_chunks tiles
                    if si in a_tiles:
                        return a_tiles[si]
                    c0 = a_chunks * si
                    c1 = min(c0 + a_chunks, p.npad // 256)
                    t = apool.tile([H, (c1 - c0) * 256], bf16, tag="aslab")
                    nc.sync.dma_start(
                        t.rearrange("p (c m) -> p c m", c=c1 - c0),
                        hA_d[d][c0:c1].rearrange("c p m -> p c m"))
                    a_tiles[si] = t
                    return t

                def emit_gs(t0, t1, gsb, d=d, p=p, psum_gs=psum_gs):
                    for T in range(t0, t1):
                        sl = touch_aslab(T // (2 * a_chunks))
                        off = T % (2 * a_chunks)
                        col = (off // 2) * 256 + (off % 2) * TILE
                        j = int(p.tile_seg[T])
                        nc.tensor.matmul(
                            psum_gs[:, j, :],
                            lhsT=sl[:, col:col + TILE],
                            rhs=gsb[:, T - t0, :],
                            start=bool(p.tile_first[T]),
                            stop=bool(p.tile_last[T]),
                        )

                # --- main tile loop, batched by gbatch for the sigmoid ---
                batches = [(t0, min(t0 + gbatch, p.ntiles))
                           for t0 in range(0, p.ntiles, gbatch)]
                prev = None
                for (t0, t1) in batches:
                    gb = t1 - t0
                    pg = gpsum.tile([H, gbatch], f32, tag="pg")
                    gsb = gsbpool.tile([H, gbatch, 2], bf16, tag="gsb")
                    nc.vector.memset(gsb[:, :, 1], 1.0)
                    for T in range(t0, t1):
                        gt = touch_bgroup(int(tile_group[T]))
                        _, _, c0, _ = bgroups[int(tile_group[T])]
                        col = T * TILE - c0
                        nc.tensor.matmul(
                            pg[:, T - t0:T - t0 + 1],
                            lhsT=gt[:, col:col + TILE],
                            rhs=wg_sb[d],
                            start=True, stop=True,
                        )
                    nc.scalar.activation(
                        gsb[:, 0:gb, 0], pg[:, 0:gb], AF.Sigmoid, bias=bg_sb[d])
                    if prev is not None:
                        emit_gs(prev[0], prev[1], prev[2])
                    prev = (t0, t1, gsb)
                emit_gs(prev[0], prev[1], prev[2])

                # --- per-rank head: pools -> r_d = agg @ Wp_d (+mean fold) ---
                gsp = persist.tile([H, segs, 2], f32, tag=f"gsp{d}")
                nc.scalar.copy(gsp, psum_gs)
                r1 = hpsum.tile([segs, H], f32, tag="r1")
                r2 = hpsum.tile([segs, H], f32, tag="r2")
                nc.tensor.matmul(r1, lhsT=gsp[:, :, 1], rhs=wp_sb[d][0], start=True, stop=False)
                nc.tensor.matmul(r1, lhsT=maxp, rhs=wp_sb[d][2], start=False, stop=False)
                nc.tensor.matmul(r1, lhsT=gsp[:, :, 0], rhs=wp_sb[d][3], start=False, stop=True)
                nc.tensor.matmul(r2, lhsT=gsp[:, :, 1], rhs=wp_sb[d][1], start=True, stop=True)
                tmp = headsb.tile([segs, H], f32, tag=f"tmp{d}")
                nc.vector.tensor_scalar_mul(tmp, r2, recip_sb[d])
                nc.vector.tensor_add(state[:, d * H:(d + 1) * H], tmp, r1)

            # --- final head ---
            st2 = headsb.tile([segs, H3], f32, tag="st2")
            nc.vector.tensor_add(st2, state, bp_sb)
            stats = headsb.tile([segs, 6], f32, tag="stats")
            nc.vector.bn_stats(out=stats, in_=st2)
            mv = headsb.tile([segs, 2], f32, tag="mv")
            nc.vector.bn_aggr(out=mv, in_=stats)
            sd = headsb.tile([segs, 1], f32, tag="sd")
            nc.scalar.activation(sd, mv[:, 1:2], AF.Sqrt, bias=eps_sb, scale=1.0)
            rstd = headsb.tile([segs, 1], f32, tag="rstd")
            nc.vector.reciprocal(out=rstd, in_=sd)
            xn = headsb.tile([segs, H3], f32, tag="xn")
            nc.vector.tensor_scalar(
                out=xn, in0=st2, scalar1=mv[:, 0:1], scalar2=rstd,
                op0=OP.subtract, op1=OP.mult)
            xg = headsb.tile([segs, H3], f32, tag="xg")
            nc.vector.tensor_mul(xg, xn, gamma_sb)
            xb = headsb.tile([segs, H3], f32, tag="xb")
            nc.vector.tensor_add(xb, xg, beta_sb)
            sg = headsb.tile([segs, H3], f32, tag="sg")
            nc.scalar.activation(sg, xb, AF.Sigmoid)
            s1 = headsb.tile([segs, H3], f32, tag="s1")
            nc.vector.tensor_mul(s1, xb, sg)

            x1 = hpsum.tile([segs, H], f32, tag="x1")
            for c in range(nranks):
                tp = hpsum.tile([H, segs], f32, tag="tp")
                nc.tensor.transpose(tp, s1[:, c * H:(c + 1) * H], id_sb)
                stT = headsb.tile([H, segs], f32, tag=f"stT{c}")
                nc.scalar.copy(stT, tp)
                nc.tensor.matmul(x1, lhsT=stT, rhs=w1_sb[c],
                                 start=(c == 0), stop=(c == nranks - 1))
            x1b = headsb.tile([segs, H], f32, tag="x1b")
            nc.vector.tensor_add(x1b, b1f_sb, x1)
            sg2 = headsb.tile([segs, H], f32, tag="sg2")
            nc.scalar.activation(sg2, x1b, AF.Sigmoid)
            x2 = headsb.tile([segs, H], f32, tag="x2")
            nc.vector.tensor_mul(x2, x1b, sg2)
            tp2 = hpsum.tile([H, segs], f32, tag="tp")
            nc.tensor.transpose(tp2, x2, id_sb)
            x2T = headsb.tile([H, segs], f32, tag="x2T")
            nc.scalar.copy(x2T, tp2)
            o_ps = hpsum.tile([segs, 1], f32, tag="x1")
            nc.tensor.matmul(o_ps, lhsT=x2T, rhs=w2_sb, start=True, stop=True)
            out_sb = headsb.tile([segs, 1], f32, tag="outsb")
            nc.scalar.activation(out_sb, o_ps, AF.Identity, bias=b2f_sb, scale=1.0)
            nc.sync.dma_start(out_t[:], out_sb)

    nc.compile()
    return nc, list(per_core.keys()), shared


# ----------------------------------------------------------------------------
# Entry point
# ----------------------------------------------------------------------------

def _prep(inputs, ncores, segs):
    """Host planning+packing. Returns (plans, consts, per_core_data)."""
    nranks = 3
    hs = [np.asarray(inputs[f"h{d}"], np.float32) for d in range(nranks)]
    bs = [np.asarray(inputs[f"b{d}"]) for d in range(nranks)]
    plans = [RankPlan(bs[d], ncores, segs) for d in range(nranks)]

    consts = {}
    for d in range(nranks):
        consts[f"wg{d}"] = np.asarray(inputs[f"Wg{d}"], np.float32).astype(BF16)
        consts[f"bg{d}"] = np.asarray(inputs[f"bg{d}"], np.float32).reshape(-1)[0]
        consts[f"wp{d}"] = np.ascontiguousarray(
            np.asarray(inputs[f"Wp{d}"], np.float32).reshape(4, H, H))
    h3 = H * nranks
    bp_cat = np.concatenate([np.asarray(inputs[f"bp{d}"], np.float32)
                             for d in range(nranks)])
    consts["bp"] = np.ascontiguousarray(np.broadcast_to(bp_cat, (segs, h3)))
    consts["gamma_b"] = np.ascontiguousarray(
        np.broadcast_to(np.asarray(inputs["gamma"], np.float32), (segs, h3)))
    consts["beta_b"] = np.ascontiguousarray(
        np.broadcast_to(np.asarray(inputs["beta"], np.float32), (segs, h3)))
    consts["w1"] = np.ascontiguousarray(
        np.asarray(inputs["W1"], np.float32).reshape(nranks, H, H))
    consts["b1f_b"] = np.ascontiguousarray(
        np.broadcast_to(np.asarray(inputs["b1f"], np.float32), (segs, H)))
    consts["w2"] = np.ascontiguousarray(np.asarray(inputs["W2"], np.float32))
    consts["b2f"] = np.asarray(inputs["b2f"], np.float32).reshape(-1)[0]
    consts["id64"] = np.eye(segs, dtype=np.float32)

    per_core = [dict() for _ in range(ncores)]
    for d in range(nranks):
        hB, hA = _pack_rank(hs[d], bs[d], plans[d])
        recip = (1.0 / np.maximum(plans[d].counts, 1.0)).astype(np.float32)
        for k in range(ncores):
            per_core[k][f"hB{d}"] = hB[k]
            per_core[k][f"hA{d}"] = hA[k]
            per_core[k][f"recip{d}"] = recip[k][:, None]
    return plans, consts, per_core


def _shim_axon_hooks():
    """This container's axon build lacks antenv.axon_hooks (NTFF profiling);
    shim it so run_bass_kernel_spmd's trace path degrades gracefully."""
    import types
    try:
        import antenv.axon_hooks  # noqa: F401
    except ImportError:
        import antenv
        m = types.ModuleType("antenv.axon_hooks")
        m.get_axon_ntff_profile_hook = lambda: None
        sys.modules["antenv.axon_hooks"] = m
        antenv.axon_hooks = m


def kernel(**inputs) -> np.ndarray:
    _shim_axon_hooks()
    from concourse.bass_utils import run_bass_kernel_spmd

    segs = B_SEGS // NCORES
    plans, consts, per_core = _prep(inputs, NCORES, segs)
    nc, pc_names, shared = build_core_program(plans, consts, segs)

    in_maps = []
    for k in range(NCORES):
        m = dict(shared)
        m.update(per_core[k])
        in_maps.append(m)

    res = run_bass_kernel_spmd(nc, in_maps, core_ids=list(range(NCORES)))
    global LAST_RESULT
    LAST_RESULT = res
    out = np.concatenate([res.results[k]["out"][:, 0] for k in range(NCORES)])
    return np.ascontiguousarray(out.astype(np.float32))


LAST_RESULT = None


if __name__ == "__main__":
    # smoke test with random data of full size
    rng = np.random.default_rng(0)
    N0 = N1 = 500_000
    N2 = 250_000
    inp = dict(
        h0=rng.standard_normal((N0, H), dtype=np.float32),
        h1=rng.standard_normal((N1, H), dtype=np.float32),
        h2=rng.standard_normal((N2, H), dtype=np.float32),
        b0=np.sort(rng.integers(0, B_SEGS, N0).astype(np.int32)),
        b1=np.sort(rng.integers(0, B_SEGS, N1).astype(np.int32)),
        b2=np.sort(rng.integers(0, B_SEGS, N2).astype(np.int32)),
    )
    for d in range(3):
        inp[f"Wg{d}"] = rng.standard_normal((H, 1), dtype=np.float32) * 0.02
        inp[f"bg{d}"] = np.zeros(1, np.float32)
        inp[f"Wp{d}"] = rng.standard_normal((4 * H, H), dtype=np.float32) * 0.02
        inp[f"bp{d}"] = np.zeros(H, np.float32)
    inp["gamma"] = np.ones(3 * H, np.float32)
    inp["beta"] = np.zeros(3 * H, np.float32)
    inp["W1"] = rng.standard_normal((3 * H, H), dtype=np.float32) * 0.02
    inp["b1f"] = np.zeros(H, np.float32)
    inp["W2"] = rng.standard_normal((H, 1), dtype=np.float32) * 0.02
    inp["b2f"] = np.zeros(1, np.float32)
    out = kernel(**inp)
    print(out.shape, out[:8])

